# revision 6
# baseline (speedup 1.0000x reference)
"""Circular rational-quadratic spline flow on 8 Trainium2 cores.

Data-parallel over the batch (131072 rows -> 16384/core). Per core:
MLP on PE (relu via ACT), spline in a rows-on-partitions layout.
Bin search + all gathers are done with masked tensor_tensor_scan tails:
state=(data+state)*W, where complement masks W zero the state at
segment boundaries, so one scan over a [kill|e_1..e_32] x 8d stream
yields per-(row,d) tail sums whose differences give every gathered
quantity.
"""

import dataclasses
import numpy as np

import concourse.bacc as bacc
import concourse.mybir as mybir
import concourse.bass_isa as bass_isa
import concourse.tile as tile
from concourse.bass_utils import run_bass_kernel_spmd

TWO_PI = 2.0 * np.pi
MIN_W = 1e-3
MIN_H = 1e-3
MIN_D = 1e-3
DERIV_SHIFT = float(np.log(np.e - 1.0))
K = 32
DH = 8
C = 64
H = 256
NCORES = 8

F32 = mybir.dt.float32
ALU = mybir.AluOpType
AX = mybir.AxisListType
ACTF = mybir.ActivationFunctionType


def _ap(ap, dims, offset_elems=0, partitions=None):
    """AP with explicit free dims [(step, count), ...] in elements."""
    p = ap.ap[0]
    if partitions is not None:
        p = [p[0], partitions]
    aps = [p] + [[s, c] for (s, c) in dims]
    return dataclasses.replace(ap, ap=aps, offset=ap.offset + offset_elems)


def _scan_raw(nc, out, data0, data1):
    """tensor_tensor_scan with multi-free-dim APs (bypasses 2D assert).
    state = (data0 + state) * data1 in AP stream order."""
    eng = nc.vector
    return eng.add_instruction(
        mybir.InstTensorScalarPtr(
            name=nc.get_next_instruction_name(),
            is_tensor_tensor_scan=True,
            is_scalar_tensor_tensor=True,
            op0=ALU.add,
            op1=ALU.mult,
            ins=[
                eng.lower_ap(data0),
                eng.lower_ap_or_imm(0.0),
                eng.lower_ap(data1),
            ],
            outs=[eng.lower_ap(out)],
        )
    )


_NC_CACHE = {}


def build_kernel(b_core):
    if b_core in _NC_CACHE:
        return _NC_CACHE[b_core]
    NQ = b_core // 128          # number of 128-row batch chunks
    GQ = min(16, NQ)            # chunks per tail group
    assert NQ % GQ == 0
    NG = NQ // GQ
    a_w = 1.0 - MIN_W * K
    a_h = 1.0 - MIN_H * K

    nc = bacc.Bacc("TRN2", debug=False)
    theta_d = nc.dram_tensor("theta", [b_core, DH], F32, kind="ExternalInput")
    xT_d = nc.dram_tensor("xT", [C, b_core], F32, kind="ExternalInput")
    w1_d = nc.dram_tensor("w1", [C, H], F32, kind="ExternalInput")
    b1_d = nc.dram_tensor("b1", [128, 2], F32, kind="ExternalInput")
    w2_d = nc.dram_tensor("w2", [H + 1, 776], F32, kind="ExternalInput")
    out_d = nc.dram_tensor("outs", [b_core, DH], F32, kind="ExternalOutput")
    lad_d = nc.dram_tensor("lad", [b_core, DH], F32, kind="ExternalOutput")
    import os as _os
    DBG = bool(int(_os.environ.get("KDBG", "0")))
    if DBG:
        dbg_ef = nc.dram_tensor("dbg_ef", [128, 2 * 257], F32, kind="ExternalOutput")
        dbg_s = nc.dram_tensor("dbg_s", [128, 264], F32, kind="ExternalOutput")
        dbg_wx = nc.dram_tensor("dbg_wx", [128, DH * 36], F32, kind="ExternalOutput")
        dbg_sc = nc.dram_tensor("dbg_sc", [128, 7 * 272], F32, kind="ExternalOutput")
        dbg_zw = nc.dram_tensor("dbg_zw", [128, DH], F32, kind="ExternalOutput")
        dbg_ix = nc.dram_tensor("dbg_ix", [128, DH], F32, kind="ExternalOutput")
        dbg_pp = nc.dram_tensor("dbg_pp", [128, 776], F32, kind="ExternalOutput")

    with tile.TileContext(nc) as tc:
        with tc.tile_pool(name="const", bufs=1) as cpool, \
             tc.tile_pool(name="w2p", bufs=1) as wpool, \
             tc.tile_pool(name="mm1", bufs=2) as mpool, \
             tc.tile_pool(name="mm1ps", bufs=2, space="PSUM") as mm1ps, \
             tc.tile_pool(name="prps", bufs=2, space="PSUM") as prps, \
             tc.tile_pool(name="chunk", bufs=3) as kpool, \
             tc.tile_pool(name="grp", bufs=2) as gpool:

            # ---------------- resident constants ----------------
            w1_t = cpool.tile([C, H], F32)
            nc.sync.dma_start(w1_t[:, :], w1_d.ap())
            b1_t = cpool.tile([128, 2], F32)
            nc.sync.dma_start(b1_t[:, :], b1_d.ap())
            w2_t = wpool.tile([128, 2, 776], F32)
            nc.sync.dma_start(
                w2_t[:, :, :],
                _ap(w2_d.ap(), [(776 * 128, 2), (1, 776)], partitions=128))
            b2row_t = cpool.tile([1, 776], F32)
            nc.sync.dma_start(
                b2row_t[:, :],
                _ap(w2_d.ap(), [(1, 776)], offset_elems=256 * 776, partitions=1))
            ones_t = cpool.tile([1, 128], F32)
            nc.vector.memset(ones_t[:, :], 1.0)
            jc_t = cpool.tile([128, K], F32)       # jc[p, k] = k
            nc.gpsimd.iota(jc_t[:, :], [[1, K]], channel_multiplier=0,
                           allow_small_or_imprecise_dtypes=True)
            # carry-kill mask for the plain cumsum: 0 at stream col 33*d
            wc_t = cpool.tile([128, 264], F32)
            nc.vector.memset(wc_t[:, :], 1.0)
            nc.vector.memset(_ap(wc_t[:, :], [(33, DH), (1, 1)]), 0.0)

            for g in range(NG):
                th_g = gpool.tile([128, GQ, DH], F32, tag="th")
                zw_g = gpool.tile([128, GQ, DH], F32, tag="zw")
                zh_g = gpool.tile([128, GQ, DH], F32, tag="zh")
                ix_g = gpool.tile([128, GQ, DH], F32, tag="ix")
                tl_g = gpool.tile([128, GQ, 7, DH], F32, tag="tl")
                nc.sync.dma_start(
                    th_g[:, :, :],
                    _ap(theta_d.ap(), [(128 * DH, GQ), (1, DH)],
                        offset_elems=g * GQ * 128 * DH, partitions=128))

                for qq in range(GQ):
                    q = g * GQ + qq
                    ql = q % 4
                    if ql == 0:
                        # -------- MM1 for the next 512 batch rows --------
                        xt_t = mpool.tile([C, 512], F32, tag="xt")
                        nc.sync.dma_start(
                            xt_t[:, :],
                            _ap(xT_d.ap(), [(1, 512)], offset_elems=q * 128))
                        ht_t = mpool.tile([128, 2, 512], F32, tag="ht")
                        for kh in range(2):
                            hps = mm1ps.tile([128, 512], F32, tag="hps")
                            nc.tensor.matmul(hps[:, :],
                                             w1_t[:, 128 * kh:128 * (kh + 1)],
                                             xt_t[:, :], start=True, stop=True)
                            nc.scalar.activation(ht_t[:, kh, :], hps[:, :],
                                                 ACTF.Relu,
                                                 bias=b1_t[:, kh:kh + 1],
                                                 scale=1.0)
                    # -------- MM2: params chunk -> PSUM [128, 776] --------
                    pps = prps.tile([128, 1024], F32, tag="pps")
                    for kh in range(2):
                        lhs = ht_t[:, kh, 128 * ql:128 * (ql + 1)]
                        nc.tensor.matmul(pps[:, 0:512], lhs, w2_t[:, kh, 0:512],
                                         start=(kh == 0), stop=False)
                        nc.tensor.matmul(pps[:, 512:776], lhs,
                                         w2_t[:, kh, 512:776],
                                         start=(kh == 0), stop=False)
                    nc.tensor.matmul(pps[:, 0:512], ones_t[:, :],
                                     b2row_t[:, 0:512], start=False, stop=True)
                    nc.tensor.matmul(pps[:, 512:776], ones_t[:, :],
                                     b2row_t[:, 512:776], start=False, stop=True)

                    # -------- spline chunk --------
                    # EF[:, a, 1+32d+k] = exp(params)  (a=0: uw, a=1: uh)
                    ef_t = kpool.tile([128, 2, 257], F32, tag="ef")
                    nc.vector.memset(_ap(ef_t[:, :, :], [(257, 2), (1, 1)]), 1.0)
                    nc.scalar.activation(
                        _ap(ef_t[:, :, :], [(257, 2), (1, 256)], offset_elems=1),
                        pps[:, 0:512], ACTF.Exp, scale=1.0)
                    nc.vector.tensor_reduce(
                        zw_g[:, qq, :],
                        _ap(ef_t[:, :, :], [(32, DH), (1, K)], offset_elems=1),
                        axis=AX.X, op=ALU.add)
                    nc.vector.tensor_reduce(
                        zh_g[:, qq, :],
                        _ap(ef_t[:, :, :], [(32, DH), (1, K)],
                            offset_elems=257 + 1),
                        axis=AX.X, op=ALU.add)
                    # local cumsum S over [kill|e]x8 stream
                    s_t = kpool.tile([128, 264], F32, tag="s")
                    _scan_raw(nc, s_t[:, :],
                              _ap(ef_t[:, :, :], [(32, DH), (1, 33)]),
                              wc_t[:, :])
                    # Tp = theta * Zw / (2pi*a)
                    tp_t = kpool.tile([128, DH], F32, tag="tp")
                    nc.vector.scalar_tensor_tensor(
                        tp_t[:, :], th_g[:, qq, :], 1.0 / (TWO_PI * a_w),
                        zw_g[:, qq, :], ALU.mult, ALU.mult)
                    # mj = (jc+1) * Zw
                    mj_t = kpool.tile([128, DH, K], F32, tag="mj")
                    nc.vector.scalar_tensor_tensor(
                        mj_t[:, :, :],
                        _ap(jc_t[:, :], [(0, DH), (1, K)]), 1.0,
                        _ap(zw_g[:, qq, :], [(1, DH), (0, K)]),
                        ALU.add, ALU.mult)
                    # R2 = Tp - mj*MIN_W/a
                    r2_t = kpool.tile([128, DH, K], F32, tag="r2")
                    nc.vector.scalar_tensor_tensor(
                        r2_t[:, :, :], mj_t[:, :, :], -MIN_W / a_w,
                        _ap(tp_t[:, :], [(1, DH), (0, K)]),
                        ALU.mult, ALU.add)
                    # Wext[d, c] = W_{c-2} = [S_local > R2]; c in {0,1,2}->0, 35->1
                    wx_t = kpool.tile([128, DH, 36], F32, tag="wx")
                    nc.vector.memset(_ap(wx_t[:, :, :], [(36, DH), (1, 3)]), 0.0)
                    nc.vector.memset(
                        _ap(wx_t[:, :, :], [(36, DH), (1, 1)], offset_elems=35),
                        1.0)
                    nc.vector.tensor_tensor(
                        _ap(wx_t[:, :, :], [(36, DH), (1, K)], offset_elems=3),
                        _ap(s_t[:, :], [(33, DH), (1, K)], offset_elems=1),
                        r2_t[:, :, :], ALU.is_gt)
                    # idx = 32 - sum(W)
                    nc.vector.tensor_reduce(
                        ix_g[:, qq, :],
                        _ap(wx_t[:, :, :], [(36, DH), (1, K)], offset_elems=3),
                        axis=AX.X, op=ALU.add)
                    # -------- 7 masked tail scans --------
                    sc_t = kpool.tile([128, 7, 272], F32, tag="sc")
                    specs = ((0, "e", 0), (1, "e", 1), (2, "f", 0), (3, "f", 1),
                             (4, "u", 0), (5, "u", 1), (6, "u", 2))
                    for (t_i, src, sh) in specs:
                        L = K if src != "u" else K + 1
                        if src == "e":
                            d0 = _ap(ef_t[:, :, :], [(32, DH), (1, 33)])
                        elif src == "f":
                            d0 = _ap(ef_t[:, :, :], [(32, DH), (1, 33)],
                                     offset_elems=257)
                        else:
                            d0 = _ap(pps[:, 0:1024], [(33, DH), (1, 34)],
                                     offset_elems=511)
                        d1 = _ap(wx_t[:, :, :], [(36, DH), (1, L + 1)],
                                 offset_elems=2 - sh)
                        _scan_raw(
                            nc,
                            _ap(sc_t[:, :, :], [(1, DH * (L + 1))],
                                offset_elems=272 * t_i),
                            d0, d1)
                    # tails -> group tile (33-stride rows, then fix udx rows)
                    nc.vector.tensor_copy(
                        tl_g[:, qq, :, :],
                        _ap(sc_t[:, :, :], [(272, 7), (33, DH)],
                            offset_elems=32))
                    nc.vector.tensor_copy(
                        tl_g[:, qq, 4:7, :],
                        _ap(sc_t[:, :, :], [(272, 3), (34, DH)],
                            offset_elems=272 * 4 + 33))
                    if DBG and q == 0:
                        ppc = kpool.tile([128, 776], F32, tag="ppc")
                        nc.vector.tensor_copy(ppc[:, :], pps[:, 0:776])
                        nc.sync.dma_start(dbg_pp.ap(), ppc[:, :])
                        nc.sync.dma_start(dbg_ef.ap(), _ap(ef_t[:, :, :], [(1, 2 * 257)]))
                        nc.sync.dma_start(dbg_s.ap(), s_t[:, :])
                        nc.sync.dma_start(dbg_wx.ap(), _ap(wx_t[:, :, :], [(1, DH * 36)]))
                        nc.sync.dma_start(dbg_sc.ap(), _ap(sc_t[:, :, :], [(1, 7 * 272)]))
                        nc.sync.dma_start(dbg_zw.ap(), zw_g[:, 0, :])
                        nc.sync.dma_start(dbg_ix.ap(), ix_g[:, 0, :])

                # ============ group tail: per-row rational quadratic ============
                def gt(tag):
                    return gpool.tile([128, GQ, DH], F32, tag=tag, name=tag)

                def f2(t):
                    return t[:, :, :]

                TT = nc.vector.tensor_tensor
                STT = nc.vector.scalar_tensor_tensor
                rzw = gt("rzw"); rzh = gt("rzh")
                nc.vector.reciprocal(f2(rzw), f2(zw_g))
                nc.vector.reciprocal(f2(rzh), f2(zh_g))
                sm0 = gt("sm0"); sm1 = gt("sm1"); hm0 = gt("hm0"); hm1 = gt("hm1")
                TT(f2(sm0), f2(zw_g), tl_g[:, :, 0, :], ALU.subtract)
                TT(f2(sm1), f2(zw_g), tl_g[:, :, 1, :], ALU.subtract)
                TT(f2(hm0), f2(zh_g), tl_g[:, :, 2, :], ALU.subtract)
                TT(f2(hm1), f2(zh_g), tl_g[:, :, 3, :], ALU.subtract)
                u1 = gt("u1"); u2 = gt("u2")
                TT(f2(u1), tl_g[:, :, 4, :], tl_g[:, :, 5, :], ALU.subtract)
                TT(f2(u2), tl_g[:, :, 5, :], tl_g[:, :, 6, :], ALU.subtract)
                icw = gt("icw"); inw = gt("inw"); ich = gt("ich"); inh = gt("inh")
                tmp = gt("tmp"); tmp2 = gt("tmp2")
                STT(f2(tmp), f2(sm0), TWO_PI * a_w, f2(rzw), ALU.mult, ALU.mult)
                STT(f2(icw), f2(ix_g), -TWO_PI * MIN_W, f2(tmp), ALU.mult, ALU.add)
                nc.vector.tensor_scalar_add(f2(icw), f2(icw), TWO_PI * MIN_W * K)
                TT(f2(tmp2), f2(sm1), f2(sm0), ALU.subtract)
                STT(f2(tmp), f2(tmp2), TWO_PI * a_w, f2(rzw), ALU.mult, ALU.mult)
                nc.vector.tensor_scalar_add(f2(inw), f2(tmp), TWO_PI * MIN_W)
                STT(f2(tmp), f2(hm0), TWO_PI * a_h, f2(rzh), ALU.mult, ALU.mult)
                STT(f2(ich), f2(ix_g), -TWO_PI * MIN_H, f2(tmp), ALU.mult, ALU.add)
                nc.vector.tensor_scalar_add(f2(ich), f2(ich), TWO_PI * MIN_H * K)
                TT(f2(tmp2), f2(hm1), f2(hm0), ALU.subtract)
                STT(f2(tmp), f2(tmp2), TWO_PI * a_h, f2(rzh), ALU.mult, ALU.mult)
                nc.vector.tensor_scalar_add(f2(inh), f2(tmp), TWO_PI * MIN_H)
                # d0/d1 = MIN_D + ln(1 + exp(u))
                e1 = gt("e1"); e2 = gt("e2"); dd0 = gt("dd0"); dd1 = gt("dd1")
                nc.scalar.activation(f2(e1), f2(u1), ACTF.Exp, scale=1.0)
                nc.scalar.activation(f2(e2), f2(u2), ACTF.Exp, scale=1.0)
                nc.vector.tensor_scalar_add(f2(e1), f2(e1), 1.0)
                nc.vector.tensor_scalar_add(f2(e2), f2(e2), 1.0)
                nc.scalar.activation(f2(dd0), f2(e1), ACTF.Ln, scale=1.0)
                nc.scalar.activation(f2(dd1), f2(e2), ACTF.Ln, scale=1.0)
                nc.vector.tensor_scalar_add(f2(dd0), f2(dd0), MIN_D)
                nc.vector.tensor_scalar_add(f2(dd1), f2(dd1), MIN_D)
                rw = gt("rw"); tt_ = gt("tt"); t1 = gt("t1")
                nc.vector.reciprocal(f2(rw), f2(inw))
                TT(f2(tmp), f2(th_g), f2(icw), ALU.subtract)
                TT(f2(tt_), f2(tmp), f2(rw), ALU.mult)
                nc.vector.tensor_scalar(f2(tmp), f2(tt_), -1.0, 1.0,
                                        ALU.mult, ALU.add)   # 1 - t
                TT(f2(t1), f2(tt_), f2(tmp), ALU.mult)
                dl = gt("dl"); t2 = gt("t2"); omt2 = gt("omt2")
                TT(f2(dl), f2(inh), f2(rw), ALU.mult)
                TT(f2(t2), f2(tt_), f2(tt_), ALU.mult)
                TT(f2(omt2), f2(tmp), f2(tmp), ALU.mult)
                nm = gt("nm"); dn = gt("dn")
                TT(f2(tmp2), f2(dl), f2(t2), ALU.mult)
                TT(f2(nm), f2(dd0), f2(t1), ALU.mult)
                TT(f2(nm), f2(nm), f2(tmp2), ALU.add)
                TT(f2(nm), f2(nm), f2(inh), ALU.mult)
                TT(f2(dn), f2(dd0), f2(dd1), ALU.add)
                STT(f2(dn), f2(dl), -2.0, f2(dn), ALU.mult, ALU.add)
                TT(f2(dn), f2(dn), f2(t1), ALU.mult)
                TT(f2(dn), f2(dn), f2(dl), ALU.add)
                rdn = gt("rdn"); outv = gt("outv")
                nc.vector.reciprocal(f2(rdn), f2(dn))
                TT(f2(outv), f2(nm), f2(rdn), ALU.mult)
                TT(f2(outv), f2(outv), f2(ich), ALU.add)
                dv = gt("dv")
                TT(f2(dv), f2(dd1), f2(t2), ALU.mult)
                STT(f2(tmp2), f2(dl), 2.0, f2(t1), ALU.mult, ALU.mult)
                TT(f2(dv), f2(dv), f2(tmp2), ALU.add)
                TT(f2(tmp2), f2(dd0), f2(omt2), ALU.mult)
                TT(f2(dv), f2(dv), f2(tmp2), ALU.add)
                TT(f2(tmp2), f2(dl), f2(dl), ALU.mult)
                TT(f2(dv), f2(dv), f2(tmp2), ALU.mult)
                ldv = gt("ldv"); ldn = gt("ldn"); ladv = gt("ladv")
                nc.scalar.activation(f2(ldv), f2(dv), ACTF.Ln, scale=1.0)
                nc.scalar.activation(f2(ldn), f2(dn), ACTF.Ln, scale=1.0)
                STT(f2(ladv), f2(ldn), -2.0, f2(ldv), ALU.mult, ALU.add)
                nc.sync.dma_start(
                    _ap(out_d.ap(), [(128 * DH, GQ), (1, DH)],
                        offset_elems=g * GQ * 128 * DH, partitions=128),
                    f2(outv))
                nc.sync.dma_start(
                    _ap(lad_d.ap(), [(128 * DH, GQ), (1, DH)],
                        offset_elems=g * GQ * 128 * DH, partitions=128),
                    f2(ladv))

    nc.compile()
    _NC_CACHE[b_core] = nc
    return nc


def prep_in_maps(theta, x_conditioner, W1, b1, W2, b2, eta):
    theta = np.ascontiguousarray(np.asarray(theta, np.float32))
    x = np.asarray(x_conditioner, np.float32)
    W1 = np.ascontiguousarray(np.asarray(W1, np.float32))
    b1 = np.asarray(b1, np.float32)
    W2 = np.asarray(W2, np.float32)
    b2 = np.asarray(b2, np.float32)
    eta = float(np.asarray(eta).reshape(-1)[0])
    B = theta.shape[0]
    bc = B // NCORES

    # host prep: W2 cols permuted to [uw(256)|uh(256)|udx(264)], * eta;
    # b2 (and DERIV_SHIFT) ride row 256 (multiplied by an on-chip ones row)
    W2e = W2 * eta
    b2e = b2 * eta
    cols = np.arange(3 * K * DH).reshape(DH, 3, K)
    uw_cols = cols[:, 0, :].reshape(-1)
    uh_cols = cols[:, 1, :].reshape(-1)
    ud_cols = cols[:, 2, :]
    udx_cols = np.concatenate([ud_cols, ud_cols[:, :1]], 1).reshape(-1)
    w2p = np.empty((H + 1, 776), np.float32)
    w2p[:H, 0:256] = W2e[:, uw_cols]
    w2p[:H, 256:512] = W2e[:, uh_cols]
    w2p[:H, 512:776] = W2e[:, udx_cols]
    w2p[H, 0:256] = b2e[uw_cols]
    w2p[H, 256:512] = b2e[uh_cols]
    w2p[H, 512:776] = b2e[udx_cols] + DERIV_SHIFT
    b1r = np.ascontiguousarray(b1.reshape(2, 128).T)

    in_maps = []
    for c in range(NCORES):
        sl = slice(c * bc, (c + 1) * bc)
        in_maps.append(dict(
            theta=theta[sl],
            xT=np.ascontiguousarray(x[sl].T),
            w1=W1, b1=b1r, w2=w2p))
    return in_maps


def kernel(theta, x_conditioner, W1, b1, W2, b2, eta):
    B = np.asarray(theta).shape[0]
    bc = B // NCORES
    nc = build_kernel(bc)
    in_maps = prep_in_maps(theta, x_conditioner, W1, b1, W2, b2, eta)
    res = run_bass_kernel_spmd(nc, in_maps, core_ids=list(range(NCORES)))
    outs = np.concatenate([r["outs"] for r in res.results], 0)
    lads = np.concatenate([r["lad"] for r in res.results], 0)
    return outs, lads


# revision 8
# speedup vs baseline: 83.1419x; 83.1419x over previous
"""Circular rational-quadratic spline flow on 8 Trainium2 cores.

Data-parallel over the batch (131072 rows -> 16384/core). Per core:
MLP on PE (relu via ACT), spline in a rows-on-partitions layout.
Bin search + all gathers are done with masked tensor_tensor_scan tails:
state=(data+state)*W, where complement masks W zero the state at
segment boundaries, so one scan over a [kill|e_1..e_32] x 8d stream
yields per-(row,d) tail sums whose differences give every gathered
quantity.
"""

import dataclasses
import numpy as np

import concourse.bacc as bacc
import concourse.mybir as mybir
import concourse.bass_isa as bass_isa
import concourse.tile as tile
from concourse.bass_utils import run_bass_kernel_spmd

TWO_PI = 2.0 * np.pi
MIN_W = 1e-3
MIN_H = 1e-3
MIN_D = 1e-3
DERIV_SHIFT = float(np.log(np.e - 1.0))
K = 32
DH = 8
C = 64
H = 256
NCORES = 8

F32 = mybir.dt.float32
ALU = mybir.AluOpType
AX = mybir.AxisListType
ACTF = mybir.ActivationFunctionType


def _ap(ap, dims, offset_elems=0, partitions=None):
    """AP with explicit free dims [(step, count), ...] in elements."""
    p = ap.ap[0]
    if partitions is not None:
        p = [p[0], partitions]
    aps = [p] + [[s, c] for (s, c) in dims]
    return dataclasses.replace(ap, ap=aps, offset=ap.offset + offset_elems)


def _scan_raw(nc, out, data0, data1):
    """tensor_tensor_scan with multi-free-dim APs (bypasses 2D assert).
    state = (data0 + state) * data1 in AP stream order."""
    eng = nc.vector
    return eng.add_instruction(
        mybir.InstTensorScalarPtr(
            name=nc.get_next_instruction_name(),
            is_tensor_tensor_scan=True,
            is_scalar_tensor_tensor=True,
            op0=ALU.add,
            op1=ALU.mult,
            ins=[
                eng.lower_ap(data0),
                eng.lower_ap_or_imm(0.0),
                eng.lower_ap(data1),
            ],
            outs=[eng.lower_ap(out)],
        )
    )


_NC_CACHE = {}


def build_kernel(b_core):
    if b_core in _NC_CACHE:
        return _NC_CACHE[b_core]
    NQ = b_core // 128          # number of 128-row batch chunks
    GQ = min(16, NQ)            # chunks per tail group
    assert NQ % GQ == 0
    NG = NQ // GQ
    a_w = 1.0 - MIN_W * K
    a_h = 1.0 - MIN_H * K

    nc = bacc.Bacc("TRN2", debug=False)
    theta_d = nc.dram_tensor("theta", [b_core, DH], F32, kind="ExternalInput")
    xT_d = nc.dram_tensor("xT", [C, b_core], F32, kind="ExternalInput")
    w1_d = nc.dram_tensor("w1", [C, H], F32, kind="ExternalInput")
    b1_d = nc.dram_tensor("b1", [128, 2], F32, kind="ExternalInput")
    w2_d = nc.dram_tensor("w2", [H + 1, 776], F32, kind="ExternalInput")
    out_d = nc.dram_tensor("outs", [b_core, DH], F32, kind="ExternalOutput")
    lad_d = nc.dram_tensor("lad", [b_core, DH], F32, kind="ExternalOutput")
    import os as _os
    DBG = bool(int(_os.environ.get("KDBG", "0")))
    if DBG:
        dbg_ef = nc.dram_tensor("dbg_ef", [128, 2 * 257], F32, kind="ExternalOutput")
        dbg_s = nc.dram_tensor("dbg_s", [128, 264], F32, kind="ExternalOutput")
        dbg_wx = nc.dram_tensor("dbg_wx", [128, DH * 36], F32, kind="ExternalOutput")
        dbg_sc = nc.dram_tensor("dbg_sc", [128, 7 * 272], F32, kind="ExternalOutput")
        dbg_zw = nc.dram_tensor("dbg_zw", [128, DH], F32, kind="ExternalOutput")
        dbg_ix = nc.dram_tensor("dbg_ix", [128, DH], F32, kind="ExternalOutput")
        dbg_pp = nc.dram_tensor("dbg_pp", [128, 776], F32, kind="ExternalOutput")

    with tile.TileContext(nc) as tc:
        with tc.tile_pool(name="const", bufs=1) as cpool, \
             tc.tile_pool(name="w2p", bufs=1) as wpool, \
             tc.tile_pool(name="mm1", bufs=2) as mpool, \
             tc.tile_pool(name="mm1ps", bufs=2, space="PSUM") as mm1ps, \
             tc.tile_pool(name="prps", bufs=2, space="PSUM") as prps, \
             tc.tile_pool(name="chunk", bufs=3) as kpool, \
             tc.tile_pool(name="grp", bufs=2) as gpool:

            # ---------------- resident constants ----------------
            w1_t = cpool.tile([C, H], F32)
            nc.sync.dma_start(w1_t[:, :], w1_d.ap())
            b1_t = cpool.tile([128, 2], F32)
            nc.sync.dma_start(b1_t[:, :], b1_d.ap())
            w2_t = wpool.tile([128, 2, 776], F32)
            nc.sync.dma_start(
                w2_t[:, :, :],
                _ap(w2_d.ap(), [(776 * 128, 2), (1, 776)], partitions=128))
            b2row_t = cpool.tile([1, 776], F32)
            nc.sync.dma_start(
                b2row_t[:, :],
                _ap(w2_d.ap(), [(1, 776)], offset_elems=256 * 776, partitions=1))
            ones_t = cpool.tile([1, 128], F32)
            nc.vector.memset(ones_t[:, :], 1.0)
            jc_t = cpool.tile([128, K], F32)       # jc[p, k] = k
            nc.gpsimd.iota(jc_t[:, :], [[1, K]], channel_multiplier=0,
                           allow_small_or_imprecise_dtypes=True)
            # carry-kill mask for the plain cumsum: 0 at stream col 33*d
            wc_t = cpool.tile([128, 264], F32)
            nc.vector.memset(wc_t[:, :], 1.0)
            nc.vector.memset(_ap(wc_t[:, :], [(33, DH), (1, 1)]), 0.0)

            for g in range(NG):
                th_g = gpool.tile([128, GQ, DH], F32, tag="th")
                zw_g = gpool.tile([128, GQ, DH], F32, tag="zw")
                zh_g = gpool.tile([128, GQ, DH], F32, tag="zh")
                ix_g = gpool.tile([128, GQ, DH], F32, tag="ix")
                tl_g = gpool.tile([128, GQ, 7, DH], F32, tag="tl")
                nc.sync.dma_start(
                    th_g[:, :, :],
                    _ap(theta_d.ap(), [(128 * DH, GQ), (1, DH)],
                        offset_elems=g * GQ * 128 * DH, partitions=128))

                for qq in range(GQ):
                    q = g * GQ + qq
                    ql = q % 4
                    if ql == 0:
                        # -------- MM1 for the next 512 batch rows --------
                        xt_t = mpool.tile([C, 512], F32, tag="xt")
                        nc.sync.dma_start(
                            xt_t[:, :],
                            _ap(xT_d.ap(), [(1, 512)], offset_elems=q * 128))
                        ht_t = mpool.tile([128, 2, 512], F32, tag="ht")
                        for kh in range(2):
                            hps = mm1ps.tile([128, 512], F32, tag="hps")
                            nc.tensor.matmul(hps[:, :],
                                             w1_t[:, 128 * kh:128 * (kh + 1)],
                                             xt_t[:, :], start=True, stop=True)
                            nc.scalar.activation(ht_t[:, kh, :], hps[:, :],
                                                 ACTF.Relu,
                                                 bias=b1_t[:, kh:kh + 1],
                                                 scale=1.0)
                    # -------- MM2: params chunk -> PSUM [128, 776] --------
                    pps = prps.tile([128, 1024], F32, tag="pps")
                    for kh in range(2):
                        lhs = ht_t[:, kh, 128 * ql:128 * (ql + 1)]
                        nc.tensor.matmul(pps[:, 0:512], lhs, w2_t[:, kh, 0:512],
                                         start=(kh == 0), stop=False)
                        nc.tensor.matmul(pps[:, 512:776], lhs,
                                         w2_t[:, kh, 512:776],
                                         start=(kh == 0), stop=False)
                    nc.tensor.matmul(pps[:, 0:512], ones_t[:, :],
                                     b2row_t[:, 0:512], start=False, stop=True)
                    nc.tensor.matmul(pps[:, 512:776], ones_t[:, :],
                                     b2row_t[:, 512:776], start=False, stop=True)

                    # -------- spline chunk --------
                    # EF[:, a, 1+32d+k] = exp(params)  (a=0: uw, a=1: uh)
                    ef_t = kpool.tile([128, 2, 257], F32, tag="ef")
                    nc.gpsimd.memset(_ap(ef_t[:, :, :], [(257, 2), (1, 1)]), 1.0)
                    nc.scalar.activation(
                        _ap(ef_t[:, :, :], [(257, 2), (1, 256)], offset_elems=1),
                        pps[:, 0:512], ACTF.Exp, scale=1.0)
                    nc.vector.tensor_reduce(
                        zw_g[:, qq, :],
                        _ap(ef_t[:, :, :], [(32, DH), (1, K)], offset_elems=1),
                        axis=AX.X, op=ALU.add)
                    nc.vector.tensor_reduce(
                        zh_g[:, qq, :],
                        _ap(ef_t[:, :, :], [(32, DH), (1, K)],
                            offset_elems=257 + 1),
                        axis=AX.X, op=ALU.add)
                    # local cumsum S over [kill|e]x8 stream
                    s_t = kpool.tile([128, 264], F32, tag="s")
                    _scan_raw(nc, s_t[:, :],
                              _ap(ef_t[:, :, :], [(32, DH), (1, 33)]),
                              wc_t[:, :])
                    # Tp = theta * Zw / (2pi*a)
                    tp_t = kpool.tile([128, DH], F32, tag="tp")
                    nc.vector.scalar_tensor_tensor(
                        tp_t[:, :], th_g[:, qq, :], 1.0 / (TWO_PI * a_w),
                        zw_g[:, qq, :], ALU.mult, ALU.mult)
                    # mj = (jc+1) * Zw
                    mj_t = kpool.tile([128, DH, K], F32, tag="mj")
                    nc.vector.scalar_tensor_tensor(
                        mj_t[:, :, :],
                        _ap(jc_t[:, :], [(0, DH), (1, K)]), 1.0,
                        _ap(zw_g[:, qq, :], [(1, DH), (0, K)]),
                        ALU.add, ALU.mult)
                    # R2 = Tp - mj*MIN_W/a
                    r2_t = kpool.tile([128, DH, K], F32, tag="r2")
                    nc.vector.scalar_tensor_tensor(
                        r2_t[:, :, :], mj_t[:, :, :], -MIN_W / a_w,
                        _ap(tp_t[:, :], [(1, DH), (0, K)]),
                        ALU.mult, ALU.add)
                    # Wext[d, c] = W_{c-2} = [S_local > R2]; c in {0,1,2}->0, 35->1
                    wx_t = kpool.tile([128, DH, 36], F32, tag="wx")
                    nc.gpsimd.memset(_ap(wx_t[:, :, :], [(36, DH), (1, 3)]), 0.0)
                    nc.gpsimd.memset(
                        _ap(wx_t[:, :, :], [(36, DH), (1, 1)], offset_elems=35),
                        1.0)
                    nc.vector.tensor_tensor(
                        _ap(wx_t[:, :, :], [(36, DH), (1, K)], offset_elems=3),
                        _ap(s_t[:, :], [(33, DH), (1, K)], offset_elems=1),
                        r2_t[:, :, :], ALU.is_gt)
                    # idx = 32 - sum(W)
                    nc.vector.tensor_reduce(
                        ix_g[:, qq, :],
                        _ap(wx_t[:, :, :], [(36, DH), (1, K)], offset_elems=3),
                        axis=AX.X, op=ALU.add)
                    # -------- 7 masked tail scans --------
                    sc_t = kpool.tile([128, 7, 272], F32, tag="sc")
                    specs = ((0, "e", 0), (1, "e", 1), (2, "f", 0), (3, "f", 1),
                             (4, "u", 0), (5, "u", 1), (6, "u", 2))
                    for (t_i, src, sh) in specs:
                        L = K if src != "u" else K + 1
                        if src == "e":
                            d0 = _ap(ef_t[:, :, :], [(32, DH), (1, 33)])
                        elif src == "f":
                            d0 = _ap(ef_t[:, :, :], [(32, DH), (1, 33)],
                                     offset_elems=257)
                        else:
                            d0 = _ap(pps[:, 0:1024], [(33, DH), (1, 34)],
                                     offset_elems=511)
                        d1 = _ap(wx_t[:, :, :], [(36, DH), (1, L + 1)],
                                 offset_elems=2 - sh)
                        _scan_raw(
                            nc,
                            _ap(sc_t[:, :, :], [(1, DH * (L + 1))],
                                offset_elems=272 * t_i),
                            d0, d1)
                    # tails -> group tile (33-stride rows, then fix udx rows)
                    nc.vector.tensor_copy(
                        tl_g[:, qq, :, :],
                        _ap(sc_t[:, :, :], [(272, 7), (33, DH)],
                            offset_elems=32))
                    nc.vector.tensor_copy(
                        tl_g[:, qq, 4:7, :],
                        _ap(sc_t[:, :, :], [(272, 3), (34, DH)],
                            offset_elems=272 * 4 + 33))
                    if DBG and q == 0:
                        ppc = kpool.tile([128, 776], F32, tag="ppc")
                        nc.vector.tensor_copy(ppc[:, :], pps[:, 0:776])
                        nc.sync.dma_start(dbg_pp.ap(), ppc[:, :])
                        nc.sync.dma_start(dbg_ef.ap(), _ap(ef_t[:, :, :], [(1, 2 * 257)]))
                        nc.sync.dma_start(dbg_s.ap(), s_t[:, :])
                        nc.sync.dma_start(dbg_wx.ap(), _ap(wx_t[:, :, :], [(1, DH * 36)]))
                        nc.sync.dma_start(dbg_sc.ap(), _ap(sc_t[:, :, :], [(1, 7 * 272)]))
                        nc.sync.dma_start(dbg_zw.ap(), zw_g[:, 0, :])
                        nc.sync.dma_start(dbg_ix.ap(), ix_g[:, 0, :])

                # ============ group tail: per-row rational quadratic ============
                def gt(tag):
                    return gpool.tile([128, GQ, DH], F32, tag=tag, name=tag)

                def f2(t):
                    return t[:, :, :]

                TT = nc.vector.tensor_tensor
                STT = nc.vector.scalar_tensor_tensor
                rzw = gt("rzw"); rzh = gt("rzh")
                nc.vector.reciprocal(f2(rzw), f2(zw_g))
                nc.vector.reciprocal(f2(rzh), f2(zh_g))
                sm0 = gt("sm0"); sm1 = gt("sm1"); hm0 = gt("hm0"); hm1 = gt("hm1")
                TT(f2(sm0), f2(zw_g), tl_g[:, :, 0, :], ALU.subtract)
                TT(f2(sm1), f2(zw_g), tl_g[:, :, 1, :], ALU.subtract)
                TT(f2(hm0), f2(zh_g), tl_g[:, :, 2, :], ALU.subtract)
                TT(f2(hm1), f2(zh_g), tl_g[:, :, 3, :], ALU.subtract)
                u1 = gt("u1"); u2 = gt("u2")
                TT(f2(u1), tl_g[:, :, 4, :], tl_g[:, :, 5, :], ALU.subtract)
                TT(f2(u2), tl_g[:, :, 5, :], tl_g[:, :, 6, :], ALU.subtract)
                icw = gt("icw"); inw = gt("inw"); ich = gt("ich"); inh = gt("inh")
                tmp = gt("tmp"); tmp2 = gt("tmp2")
                STT(f2(tmp), f2(sm0), TWO_PI * a_w, f2(rzw), ALU.mult, ALU.mult)
                STT(f2(icw), f2(ix_g), -TWO_PI * MIN_W, f2(tmp), ALU.mult, ALU.add)
                nc.vector.tensor_scalar_add(f2(icw), f2(icw), TWO_PI * MIN_W * K)
                TT(f2(tmp2), f2(sm1), f2(sm0), ALU.subtract)
                STT(f2(tmp), f2(tmp2), TWO_PI * a_w, f2(rzw), ALU.mult, ALU.mult)
                nc.vector.tensor_scalar_add(f2(inw), f2(tmp), TWO_PI * MIN_W)
                STT(f2(tmp), f2(hm0), TWO_PI * a_h, f2(rzh), ALU.mult, ALU.mult)
                STT(f2(ich), f2(ix_g), -TWO_PI * MIN_H, f2(tmp), ALU.mult, ALU.add)
                nc.vector.tensor_scalar_add(f2(ich), f2(ich), TWO_PI * MIN_H * K)
                TT(f2(tmp2), f2(hm1), f2(hm0), ALU.subtract)
                STT(f2(tmp), f2(tmp2), TWO_PI * a_h, f2(rzh), ALU.mult, ALU.mult)
                nc.vector.tensor_scalar_add(f2(inh), f2(tmp), TWO_PI * MIN_H)
                # d0/d1 = MIN_D + ln(1 + exp(u))
                e1 = gt("e1"); e2 = gt("e2"); dd0 = gt("dd0"); dd1 = gt("dd1")
                nc.scalar.activation(f2(e1), f2(u1), ACTF.Exp, scale=1.0)
                nc.scalar.activation(f2(e2), f2(u2), ACTF.Exp, scale=1.0)
                nc.vector.tensor_scalar_add(f2(e1), f2(e1), 1.0)
                nc.vector.tensor_scalar_add(f2(e2), f2(e2), 1.0)
                nc.scalar.activation(f2(dd0), f2(e1), ACTF.Ln, scale=1.0)
                nc.scalar.activation(f2(dd1), f2(e2), ACTF.Ln, scale=1.0)
                nc.vector.tensor_scalar_add(f2(dd0), f2(dd0), MIN_D)
                nc.vector.tensor_scalar_add(f2(dd1), f2(dd1), MIN_D)
                rw = gt("rw"); tt_ = gt("tt"); t1 = gt("t1")
                nc.vector.reciprocal(f2(rw), f2(inw))
                TT(f2(tmp), f2(th_g), f2(icw), ALU.subtract)
                TT(f2(tt_), f2(tmp), f2(rw), ALU.mult)
                nc.vector.tensor_scalar(f2(tmp), f2(tt_), -1.0, 1.0,
                                        ALU.mult, ALU.add)   # 1 - t
                TT(f2(t1), f2(tt_), f2(tmp), ALU.mult)
                dl = gt("dl"); t2 = gt("t2"); omt2 = gt("omt2")
                TT(f2(dl), f2(inh), f2(rw), ALU.mult)
                TT(f2(t2), f2(tt_), f2(tt_), ALU.mult)
                TT(f2(omt2), f2(tmp), f2(tmp), ALU.mult)
                nm = gt("nm"); dn = gt("dn")
                TT(f2(tmp2), f2(dl), f2(t2), ALU.mult)
                TT(f2(nm), f2(dd0), f2(t1), ALU.mult)
                TT(f2(nm), f2(nm), f2(tmp2), ALU.add)
                TT(f2(nm), f2(nm), f2(inh), ALU.mult)
                TT(f2(dn), f2(dd0), f2(dd1), ALU.add)
                STT(f2(dn), f2(dl), -2.0, f2(dn), ALU.mult, ALU.add)
                TT(f2(dn), f2(dn), f2(t1), ALU.mult)
                TT(f2(dn), f2(dn), f2(dl), ALU.add)
                rdn = gt("rdn"); outv = gt("outv")
                nc.vector.reciprocal(f2(rdn), f2(dn))
                TT(f2(outv), f2(nm), f2(rdn), ALU.mult)
                TT(f2(outv), f2(outv), f2(ich), ALU.add)
                dv = gt("dv")
                TT(f2(dv), f2(dd1), f2(t2), ALU.mult)
                STT(f2(tmp2), f2(dl), 2.0, f2(t1), ALU.mult, ALU.mult)
                TT(f2(dv), f2(dv), f2(tmp2), ALU.add)
                TT(f2(tmp2), f2(dd0), f2(omt2), ALU.mult)
                TT(f2(dv), f2(dv), f2(tmp2), ALU.add)
                TT(f2(tmp2), f2(dl), f2(dl), ALU.mult)
                TT(f2(dv), f2(dv), f2(tmp2), ALU.mult)
                ldv = gt("ldv"); ldn = gt("ldn"); ladv = gt("ladv")
                nc.scalar.activation(f2(ldv), f2(dv), ACTF.Ln, scale=1.0)
                nc.scalar.activation(f2(ldn), f2(dn), ACTF.Ln, scale=1.0)
                STT(f2(ladv), f2(ldn), -2.0, f2(ldv), ALU.mult, ALU.add)
                nc.sync.dma_start(
                    _ap(out_d.ap(), [(128 * DH, GQ), (1, DH)],
                        offset_elems=g * GQ * 128 * DH, partitions=128),
                    f2(outv))
                nc.sync.dma_start(
                    _ap(lad_d.ap(), [(128 * DH, GQ), (1, DH)],
                        offset_elems=g * GQ * 128 * DH, partitions=128),
                    f2(ladv))

    nc.compile()
    _NC_CACHE[b_core] = nc
    return nc


def prep_in_maps(theta, x_conditioner, W1, b1, W2, b2, eta):
    theta = np.ascontiguousarray(np.asarray(theta, np.float32))
    x = np.asarray(x_conditioner, np.float32)
    W1 = np.ascontiguousarray(np.asarray(W1, np.float32))
    b1 = np.asarray(b1, np.float32)
    W2 = np.asarray(W2, np.float32)
    b2 = np.asarray(b2, np.float32)
    eta = float(np.asarray(eta).reshape(-1)[0])
    B = theta.shape[0]
    bc = B // NCORES

    # host prep: W2 cols permuted to [uw(256)|uh(256)|udx(264)], * eta;
    # b2 (and DERIV_SHIFT) ride row 256 (multiplied by an on-chip ones row)
    W2e = W2 * eta
    b2e = b2 * eta
    cols = np.arange(3 * K * DH).reshape(DH, 3, K)
    uw_cols = cols[:, 0, :].reshape(-1)
    uh_cols = cols[:, 1, :].reshape(-1)
    ud_cols = cols[:, 2, :]
    udx_cols = np.concatenate([ud_cols, ud_cols[:, :1]], 1).reshape(-1)
    w2p = np.empty((H + 1, 776), np.float32)
    w2p[:H, 0:256] = W2e[:, uw_cols]
    w2p[:H, 256:512] = W2e[:, uh_cols]
    w2p[:H, 512:776] = W2e[:, udx_cols]
    w2p[H, 0:256] = b2e[uw_cols]
    w2p[H, 256:512] = b2e[uh_cols]
    w2p[H, 512:776] = b2e[udx_cols] + DERIV_SHIFT
    b1r = np.ascontiguousarray(b1.reshape(2, 128).T)

    in_maps = []
    for c in range(NCORES):
        sl = slice(c * bc, (c + 1) * bc)
        in_maps.append(dict(
            theta=theta[sl],
            xT=np.ascontiguousarray(x[sl].T),
            w1=W1, b1=b1r, w2=w2p))
    return in_maps


def kernel(theta, x_conditioner, W1, b1, W2, b2, eta):
    B = np.asarray(theta).shape[0]
    bc = B // NCORES
    nc = build_kernel(bc)
    in_maps = prep_in_maps(theta, x_conditioner, W1, b1, W2, b2, eta)
    res = run_bass_kernel_spmd(nc, in_maps, core_ids=list(range(NCORES)))
    outs = np.concatenate([r["outs"] for r in res.results], 0)
    lads = np.concatenate([r["lad"] for r in res.results], 0)
    return outs, lads
